# revision 1
# baseline (speedup 1.0000x reference)
"""CrossSparseWindowAttention Trainium2 kernel.

Sharding: pure data parallelism over batch (B=8 -> 8 cores, 1 image each).
Per-core layout strategy:
  - tokens processed in "bands" of 8 image rows (= 1 window-row = W/8 windows)
  - x band DMA'd window-major: partition p = (w%2)*64 + r*8 + c, tile = w//2
  - x cast to bf16 (gpsimd), transposed to channel-major xT [97, TB] via DVE
    32x32 stream transposes (row 96 = ones for bias folding)
  - qT,kT: Form1 matmuls (lhsT = weight chunk, rhs = xT) -> channel-major
  - v: Form2 matmuls (lhsT = xT tile, rhs = weights) -> token-major
  - window means via small matmuls vs block-ones; global kv from gt
  - S.T per (window w, head h): lhsT = kT slice, rhs = qT slice; packed
    16 slots per PSUM bank via tile_position; bias add (DVE) + exp (ACT)
  - global sims packed in own banks (rows 32*(w%4))
  - O natural: lhsT = P.T, rhs = [v | ones]; row sums via ones column
  - normalize via DVE reciprocal + broadcast multiply into f32 slabs
  - local+global merged by accumulating PE transposes; out-proj; DMA out
"""

import sys

if "/opt/trn_rl_repo" not in sys.path:
    sys.path.insert(0, "/opt/trn_rl_repo")

import numpy as np
import ml_dtypes

import concourse.bass as bass
import concourse.mybir as mybir
from concourse import bacc
from concourse.tile import TileContext
from concourse.bass_utils import run_bass_kernel_spmd

F32 = mybir.dt.float32
BF16 = mybir.dt.bfloat16
AF = mybir.ActivationFunctionType
ALU = mybir.AluOpType

# problem constants (hardcoded per spec)
B, H, W, C = 8, 256, 256, 96
M = 8          # window size
HD = 32        # head dim
NH = 3         # heads
G = 2          # global tokens
MM = M * M     # 64 tokens / window
ALPHA = 0.25

N_CORES = 8


def _rel_pos_index(m):
    coords = np.stack(np.meshgrid(np.arange(m), np.arange(m), indexing="ij"))
    flat = coords.reshape(2, -1)
    rel = (flat[:, :, None] - flat[:, None, :]).transpose(1, 2, 0).copy()
    rel[:, :, 0] += m - 1
    rel[:, :, 1] += m - 1
    rel[:, :, 0] *= 2 * m - 1
    return rel.sum(-1)  # [MM, MM]


def build_nc(h=H, w=W):
    """Build the per-core Bass program for an [h, w, C] image."""
    assert h % M == 0 and w % 16 == 0
    WB = w // M          # windows per band
    TB = MM * WB         # tokens per band
    TPB = TB // 128      # 128-token tiles per band
    WT = w // 16         # x-dma tiles per band (2 windows each) == TPB
    NBAND = h // M
    NSPREAD = WB // 4    # v_g spread groups per band
    QKCH = max(1, TB // 512)  # 512-token chunks for q/k projection
    QCH = TB // QKCH

    nc = bacc.Bacc("TRN2", target_bir_lowering=False, debug=False,
                   num_devices=N_CORES)

    # x/y are pre-permuted host-side to window-major token order:
    # token = (wrow, wcol, r, c) flattened
    x = nc.declare_dram_parameter("x", [h * w, C], F32, isOutput=False)
    y = nc.declare_dram_parameter("y", [h * w, C], F32, isOutput=True)
    w_q = nc.declare_dram_parameter("w_q_aug", [C + 1, C], BF16, isOutput=False)
    w_k = nc.declare_dram_parameter("w_k_aug", [C + 1, C], BF16, isOutput=False)
    w_v = nc.declare_dram_parameter("w_v_aug", [C + 1, C], BF16, isOutput=False)
    w_gk = nc.declare_dram_parameter("w_gk", [C, C], BF16, isOutput=False)
    w_gv = nc.declare_dram_parameter("w_gv", [C, C], BF16, isOutput=False)
    w_o = nc.declare_dram_parameter("w_out_aug", [C + 1, C], BF16, isOutput=False)
    gtokT_d = nc.declare_dram_parameter("gtokT", [C, G], BF16, isOutput=False)
    bones_d = nc.declare_dram_parameter("blockones", [128, 2], F32, isOutput=False)
    ident_d = nc.declare_dram_parameter("ident", [128, 128], F32, isOutput=False)
    tmpl_d = nc.declare_dram_parameter("bias_tmpl", [NH, 128, 512], F32, isOutput=False)

    with TileContext(nc) as tc:
        with (
            tc.tile_pool(name="const", bufs=1) as cpool,
            tc.tile_pool(name="slab", bufs=2) as spool,
            tc.tile_pool(name="pslab", bufs=3) as ppool,
            tc.tile_pool(name="psum", bufs=1, space="PSUM") as psum,
        ):
            # ---- load constants to SBUF
            wq_sb = cpool.tile([C + 1, C], BF16, name="wq_sb")
            wk_sb = cpool.tile([C + 1, C], BF16, name="wk_sb")
            wv_sb = cpool.tile([C + 1, C], BF16, name="wv_sb")
            wgk_sb = cpool.tile([C, C], BF16, name="wgk_sb")
            wgv_sb = cpool.tile([C, C], BF16, name="wgv_sb")
            wo_sb = cpool.tile([C + 1, C], BF16, name="wo_sb")
            gtok_sb = cpool.tile([C, G], BF16, name="gtok_sb")
            bones_sb = cpool.tile([128, 2], F32, name="bones_sb")
            ident_sb = cpool.tile([128, 128], F32, name="ident_sb")
            tmpl_sb = cpool.tile([128, NH, 512], F32, name="tmpl_sb")
            for dst, src in ((wq_sb, w_q), (wk_sb, w_k), (wv_sb, w_v),
                             (wgk_sb, w_gk), (wgv_sb, w_gv), (wo_sb, w_o),
                             (gtok_sb, gtokT_d), (bones_sb, bones_d),
                             (ident_sb, ident_d)):
                nc.sync.dma_start(dst[:], src[:])
            nc.sync.dma_start(
                tmpl_sb[:],
                tmpl_d.rearrange("n p f -> p n f"))

            for b in range(NBAND):
                # tokens already window-major: partition p = (w%2)*64 + r*8+c
                xin = x[b * TB:(b + 1) * TB].rearrange("(f p) ch -> p f ch",
                                                       p=128)
                x_sb = spool.tile([128, WT, C], F32, name="x_sb")
                nc.sync.dma_start(x_sb[:], xin)
                x_bf = spool.tile([128, WT, C], BF16, name="x_bf")
                nc.gpsimd.tensor_copy(x_bf[:], x_sb[:])

                # ---- transpose x -> xT [C+1, TB] bf16 (stream 32x32 blocks)
                xT = spool.tile([C + 1, TB], BF16, name="xT")
                nc.gpsimd.memset(xT[C:C + 1, :], 1.0)
                xT3 = xT.rearrange("p (t q) -> p t q", q=128)
                for pi in range(4):
                    for cj in range(3):
                        nc.vector.transpose(
                            xT3[32 * cj:32 * cj + 32, :, 32 * pi:32 * pi + 32],
                            x_bf[32 * pi:32 * pi + 32, :, 32 * cj:32 * cj + 32])

                # ---- q/k projection (Form1, channel-major out)
                qT = spool.tile([C, TB], BF16, name="qT")
                kcomb = spool.tile([C, (MM + G) * WB], BF16, name="kcomb")
                for t in range(QKCH):
                    q_ps = psum.tile([128, 512], F32, name="q_ps", tag="pa", bufs=4)
                    k_ps = psum.tile([128, 512], F32, name="k_ps", tag="pa", bufs=4)
                    rhs = xT[:, t * QCH:(t + 1) * QCH]
                    nc.tensor.matmul(q_ps[:C, :QCH], wq_sb[:], rhs)
                    nc.tensor.matmul(k_ps[:C, :QCH], wk_sb[:], rhs)
                    nc.vector.tensor_copy(qT[:, t * QCH:(t + 1) * QCH],
                                          q_ps[:C, :QCH])
                    nwc = QCH // MM
                    kc3 = kcomb.rearrange("p (w s) -> p w s", s=MM + G)
                    nc.scalar.copy(
                        kc3[:, t * nwc:(t + 1) * nwc, :MM],
                        k_ps[:C, :QCH].rearrange("p (w s) -> p w s", s=MM))

                # ---- v projection (Form2, token-major out) + window sums
                v_slab = spool.tile([128, TPB, NH * (MM // 2 + 1)], BF16,
                                    name="v_slab")
                v4 = v_slab.rearrange("p t (nh s) -> p t nh s", s=MM // 2 + 1)
                nc.gpsimd.memset(v4[:, :, :, 32:33], 1.0)
                mean_ps = psum.tile([128, 512], F32, name="mean_ps", tag="pa",
                                    bufs=4)
                for j in range(TPB):
                    v_ps = psum.tile([128, 512], F32, name="v_ps", tag="pa",
                                     bufs=4)
                    nc.tensor.matmul(v_ps[:, :C], xT[:, 128 * j:128 * (j + 1)],
                                     wv_sb[:])
                    nc.vector.tensor_copy(
                        v4[:, j, :, :32],
                        v_ps[:, :C].rearrange("p (nh s) -> p nh s", s=32))
                    nc.tensor.matmul(mean_ps[:C, 2 * j:2 * j + 2],
                                     x_sb[:, j, :], bones_sb[:])

                # ---- gt = gtok + mean (spread layout), global k/v
                gtT = spool.tile([C, NSPREAD * 128], BF16, name="gtT")
                nc.gpsimd.memset(gtT[:], 0.0)
                gt4 = gtT.rearrange("p (n w g2) -> p n w g2", w=4, g2=32)
                nc.vector.tensor_tensor(
                    gt4[:, :, :, 0:G],
                    mean_ps[:C, 0:WB].rearrange(
                        "p (n w) -> p n w", w=4
                    )[:, :, :, None].to_broadcast([C, NSPREAD, 4, G]),
                    gtok_sb[:, None, None, :].to_broadcast([C, NSPREAD, 4, G]),
                    ALU.add)

                kg_ps = psum.tile([128, 512], F32, name="kg_ps", tag="pa", bufs=4)
                nc.tensor.matmul(kg_ps[:C, :WB * G], wgk_sb[:],
                                 gt4[:, :, :, 0:G])
                kc3 = kcomb.rearrange("p (w s) -> p w s", s=MM + G)
                nc.scalar.copy(
                    kc3[:, :, MM:MM + G],
                    kg_ps[:C, :WB * G].rearrange("p (w g) -> p w g", g=G))

                vg_slab = spool.tile([128, NSPREAD, NH * 33], BF16, name="vg_slab")
                vg4 = vg_slab.rearrange("p n (nh s) -> p n nh s", s=33)
                nc.gpsimd.memset(vg4[:, :, :, 32:33], 1.0)
                for s in range(NSPREAD):
                    vg_ps = psum.tile([128, 512], F32, name="vg_ps", tag="pa",
                                      bufs=4)
                    nc.tensor.matmul(vg_ps[:, :C],
                                     gtT[:, 128 * s:128 * (s + 1)], wgv_sb[:])
                    nc.vector.tensor_copy(
                        vg4[:, s, :, :32],
                        vg_ps[:, :C].rearrange("p (nh s) -> p nh s", s=32))

                # ---- attention per head
                OL = spool.tile([128, TPB, C], F32, name="OL")
                OG = spool.tile([128, TPB, C], F32, name="OG")
                NBK = (WB + 15) // 16       # S banks per head
                NGB = (WB + 31) // 32       # G banks per head
                for hh in range(NH):
                    hp = slice(32 * hh, 32 * hh + 32)
                    # S banks: 16 windows each
                    pslabs = []
                    for bk in range(NBK):
                        s_ps = psum.tile([128, 512], F32, name="s_ps",
                                         tag="pb", bufs=2)
                        for wi in range(16 * bk, min(16 * bk + 16, WB)):
                            sl = wi % 16
                            pp, qq = sl % 2, sl // 2
                            nc.tensor.matmul(
                                s_ps[64 * pp:64 * pp + 64,
                                     64 * qq:64 * qq + 64],
                                kcomb[hp, (MM + G) * wi:(MM + G) * wi + MM],
                                qT[hp, MM * wi:MM * wi + MM],
                                tile_position=(32 * hh, 64 * pp))
                        nc.vector.tensor_tensor(s_ps[:], s_ps[:],
                                                tmpl_sb[:, hh, :], ALU.add)
                        p_sb = ppool.tile([128, 512], BF16, name="p_sb",
                                          tag="p_sb", bufs=3)
                        nc.scalar.activation(p_sb[:], s_ps[:], AF.Exp)
                        pslabs.append(p_sb)
                    # G banks: 32 windows each, rows 32*(w%4)
                    pgslabs = []
                    for bk in range(NGB):
                        g_ps = psum.tile([128, 512], F32, name="g_ps",
                                         tag="pb", bufs=2)
                        for wi in range(32 * bk, min(32 * bk + 32, WB)):
                            sl = wi % 32
                            ii, qq = sl % 4, sl // 4
                            nc.tensor.matmul(
                                g_ps[32 * ii:32 * ii + G,
                                     64 * qq:64 * qq + 64],
                                kcomb[hp, (MM + G) * wi + MM:(MM + G) * (wi + 1)],
                                qT[hp, MM * wi:MM * wi + MM],
                                tile_position=(32 * hh, 32 * ii))
                        pg_sb = ppool.tile([128, 512], BF16, name="pg_sb",
                                           tag="pg_sb", bufs=2)
                        for ii in range(4):
                            nc.scalar.activation(
                                pg_sb[32 * ii:32 * ii + G, :],
                                g_ps[32 * ii:32 * ii + G, :], AF.Exp)
                        pgslabs.append(pg_sb)
                    # O banks: 7 window-pairs each
                    NP = TPB  # window pairs per band
                    t0 = 0
                    while t0 < NP:
                        npair = min(7, NP - t0)
                        o_ps = psum.tile([128, 512], F32, name="o_ps",
                                         tag="pc", bufs=2)
                        for j in range(npair):
                            t = t0 + j
                            w0, w1 = 2 * t, 2 * t + 1
                            ps0 = pslabs[w0 // 16]
                            q0 = (w0 % 16) // 2
                            ps1 = pslabs[w1 // 16]
                            q1 = (w1 % 16) // 2
                            # local even / odd
                            nc.tensor.matmul(
                                o_ps[0:64, 66 * j:66 * j + 33],
                                ps0[0:64, 64 * q0:64 * q0 + 64],
                                v_slab[0:64, t, 33 * hh:33 * hh + 33],
                                tile_position=(0, 0))
                            nc.tensor.matmul(
                                o_ps[64:128, 66 * j:66 * j + 33],
                                ps1[64:128, 64 * q1:64 * q1 + 64],
                                v_slab[64:128, t, 33 * hh:33 * hh + 33],
                                tile_position=(64, 64))
                            # global even / odd
                            pg0 = pgslabs[w0 // 32]
                            i0, g0 = w0 % 4, (w0 % 32) // 4
                            pg1 = pgslabs[w1 // 32]
                            i1, g1 = w1 % 4, (w1 % 32) // 4
                            nc.tensor.matmul(
                                o_ps[0:64, 66 * j + 33:66 * j + 66],
                                pg0[32 * i0:32 * i0 + G,
                                    64 * g0:64 * g0 + 64],
                                vg_slab[32 * i0:32 * i0 + G, w0 // 4,
                                        33 * hh:33 * hh + 33],
                                tile_position=(32 * i0, 0))
                            nc.tensor.matmul(
                                o_ps[64:128, 66 * j + 33:66 * j + 66],
                                pg1[32 * i1:32 * i1 + G,
                                    64 * g1:64 * g1 + 64],
                                vg_slab[32 * i1:32 * i1 + G, w1 // 4,
                                        33 * hh:33 * hh + 33],
                                tile_position=(32 * i1, 64))
                        # normalize bank -> OL / OG slabs
                        sinv = ppool.tile([128, 16], F32, name="sinv",
                                          tag="sinv", bufs=3)
                        o4 = o_ps[:, :462].rearrange(
                            "p (j two u) -> p j two u", two=2, u=33)
                        s3 = sinv.rearrange("p (j two) -> p j two", two=2)
                        nc.vector.reciprocal(s3[:, :npair, :],
                                             o4[:, :npair, :, 32])
                        nc.vector.tensor_tensor(
                            OL[:, t0:t0 + npair, 32 * hh:32 * hh + 32],
                            o4[:, :npair, 0, :32],
                            s3[:, :npair, 0:1].to_broadcast([128, npair, 32]),
                            ALU.mult)
                        nc.vector.tensor_tensor(
                            OG[:, t0:t0 + npair, 32 * hh:32 * hh + 32],
                            o4[:, :npair, 1, :32],
                            s3[:, :npair, 1:2].to_broadcast([128, npair, 32]),
                            ALU.mult)
                        t0 += npair

                # ---- merge local+global via accumulating PE transpose,
                #      out-projection, write out
                olhsT = spool.tile([C + 1, TB], BF16, name="olhsT")
                nc.gpsimd.memset(olhsT[C:C + 1, :], 1.0)
                y_sb = spool.tile([128, TPB, C], F32, name="y_sb")
                for j in range(TPB):
                    ot_ps = psum.tile([128, 512], F32, name="ot_ps",
                                      tag="pc", bufs=2)
                    nc.tensor.matmul(ot_ps[:C, :128], OL[:, j, :], ident_sb[:],
                                     is_transpose=True, start=True, stop=False)
                    nc.tensor.matmul(ot_ps[:C, :128], OG[:, j, :], ident_sb[:],
                                     is_transpose=True, start=False, stop=True)
                    nc.scalar.copy(olhsT[:C, 128 * j:128 * (j + 1)],
                                   ot_ps[:C, :128])
                    yp = psum.tile([128, 512], F32, name="yp", tag="pc", bufs=2)
                    nc.tensor.matmul(yp[:, :C],
                                     olhsT[:, 128 * j:128 * (j + 1)], wo_sb[:])
                    nc.scalar.copy(y_sb[:, j, :], yp[:, :C])
                yout = y[b * TB:(b + 1) * TB].rearrange("(f p) ch -> p f ch",
                                                        p=128)
                nc.sync.dma_start(yout, y_sb[:])

    nc.compile()
    return nc


_NC_CACHE = {}


def _get_nc(h, w):
    key = (h, w)
    if key not in _NC_CACHE:
        _NC_CACHE[key] = build_nc(h, w)
    return _NC_CACHE[key]


def prep_consts(w_qkv, b_qkv, global_tokens, w_gkv, rel_bias_table, w_out,
                b_out):
    scale = np.float32(HD ** -0.5)
    w_qkv = np.asarray(w_qkv, np.float32)
    b_qkv = np.asarray(b_qkv, np.float32)
    wq = np.concatenate([w_qkv[:, 0:C], b_qkv[None, 0:C]], 0)
    wk = np.concatenate([w_qkv[:, C:2 * C], b_qkv[None, C:2 * C]], 0) * scale
    wv = np.concatenate([w_qkv[:, 2 * C:], b_qkv[None, 2 * C:]], 0) * (1 - ALPHA)
    w_gkv = np.asarray(w_gkv, np.float32)
    wgk = w_gkv[:, 0:C]
    wgv = w_gkv[:, C:] * ALPHA
    wo = np.concatenate([np.asarray(w_out, np.float32),
                         np.asarray(b_out, np.float32)[None, :]], 0)
    gtokT = np.asarray(global_tokens, np.float32)[0].T.copy()  # [C, G]

    rel = _rel_pos_index(M)                       # [MM, MM]
    tbl = np.asarray(rel_bias_table, np.float32)  # [(2M-1)^2, NH]
    bias = tbl[rel.reshape(-1)].reshape(MM, MM, NH)  # [q, k, h]
    tmpl = np.zeros((NH, 128, 512), np.float32)
    for hh in range(NH):
        bt = bias[:, :, hh].T  # [k, q]
        tmpl[hh] = np.tile(bt, (2, 8))

    bones = np.zeros((128, 2), np.float32)
    bones[0:64, 0] = 1.0 / MM
    bones[64:128, 1] = 1.0 / MM
    ident = np.eye(128, dtype=np.float32)

    bf = ml_dtypes.bfloat16
    return {
        "w_q_aug": wq.astype(bf), "w_k_aug": wk.astype(bf),
        "w_v_aug": wv.astype(bf), "w_gk": wgk.astype(bf),
        "w_gv": wgv.astype(bf), "w_out_aug": wo.astype(bf),
        "gtokT": gtokT.astype(bf), "blockones": bones,
        "ident": ident, "bias_tmpl": tmpl,
    }


def kernel(x, w_qkv, b_qkv, global_tokens, w_gkv, rel_bias_table, w_out,
           b_out, window_size, head_dim):
    assert int(window_size) == M and int(head_dim) == HD
    x = np.asarray(x, np.float32)
    b, h, w, c = x.shape
    assert c == C
    consts = prep_consts(w_qkv, b_qkv, global_tokens, w_gkv, rel_bias_table,
                         w_out, b_out)
    n = min(N_CORES, b)
    assert b % n == 0
    per = b // n
    nc = _get_nc(per * h, w)
    # host-side window-major permutation (pure data movement):
    # [b, h, w, C] -> per core [per*h*w, C] with token = (wrow, wcol, r, c)
    nh, nw = h // M, w // M
    x_wm = x.reshape(b, nh, M, nw, M, c).transpose(0, 1, 3, 2, 4, 5)
    x_wm = np.ascontiguousarray(x_wm).reshape(n, per * h * w, c)
    in_maps = [dict(consts, x=x_wm[i]) for i in range(n)]
    res = run_bass_kernel_spmd(nc, in_maps, core_ids=list(range(n)))
    y_wm = np.stack([res.results[i]["y"] for i in range(n)])
    out = y_wm.reshape(b, nh, nw, M, M, c).transpose(0, 1, 3, 2, 4, 5)
    return np.ascontiguousarray(out).reshape(b, h, w, c)

